# revision 2
# baseline (speedup 1.0000x reference)
"""Trainium2 Bass kernel for the AttentionBlock problem — v3.

Baseline (fp16 hi/lo) numerics with three performance fixes:

1. Projections are 2 fp16 matmuls instead of fp32 (which lowers to 2
   half-speed passes each, ~2.1us/MM cold): the host pre-splits x into
   [x_hi; x_lo] fp16 (xT_hl, same 2MB as fp32 xT) and R/8, E into hi/lo:
     Q = R_hi*(x_hi + x_lo) + R_lo*x_hi   (drops R_lo*x_lo ~ 2^-22)
   MM1: lhsT=[R_hi;R_hi](128x64) rhs=xT_hl, MM2: lhsT=R_lo(64x64) rhs=x_hi,
   accumulated in one PSUM bank.
2. PE warmup: ~20 junk matmuls from ~0.5us keep the PE busy through the
   DMA-init dead time so the HAM clock gate reaches 2.4GHz early, and the
   projection/pass-1 phase no longer starves (it was DMA+fp32-bound).
3. DMA layout: q-side + small tensors on the scalar HW queue (parallel to
   sync's xT stream), xaug in 4 pieces, last-chunk output split across
   sync+scalar.

Everything else (pass-1 fp16-hi max, S^T hi/lo 2-MM with the ones/-m row,
fp16 PV with augmented x, transpose+reciprocal normalize) is the proven
baseline structure.
"""

import numpy as np
from contextlib import ExitStack

import concourse.bass as bass
import concourse.tile as tile
from concourse import bacc, mybir

N = 8192
D = 64
DA = D + 1
NCORES = 8
NQ = N // NCORES          # 1024 queries per core
NKB = N // 128            # 64 key blocks
NSC = N // 512            # 16 key chunks of 512
NKQ = 8                   # kt tile count (1024 keys each)
KW = N // NKQ             # 1024
QC = 512                  # query chunk (pass-2 moving dim)
NQC = NQ // QC            # 2
NRT = QC // 128           # row-tiles per chunk (4)
NG = N // 1024            # pass-1 reduce groups per row-tile (8)
WARM = 20                 # PE warmup junk matmuls
DP = 72                   # x_aug block stride, 16-byte aligned in fp16

F32 = mybir.dt.float32
F16 = mybir.dt.float16


def build():
    nc = bacc.Bacc("TRN2", target_bir_lowering=False, debug=False, num_devices=1)

    xhl_ap = nc.dram_tensor("xT_hl", [128, N], F16, kind="ExternalInput").ap()
    xqhl_ap = nc.dram_tensor("xqT_hl", [128, NQ], F16, kind="ExternalInput").ap()
    rhh_ap = nc.dram_tensor("Rhh", [128, D], F16, kind="ExternalInput").ap()
    rlo_ap = nc.dram_tensor("Rlo", [D, D], F16, kind="ExternalInput").ap()
    ehh_ap = nc.dram_tensor("Ehh", [128, D], F16, kind="ExternalInput").ap()
    elo_ap = nc.dram_tensor("Elo", [D, D], F16, kind="ExternalInput").ap()
    id_ap = nc.dram_tensor("ident", [128, 128], F32, kind="ExternalInput").ap()
    ones16_ap = nc.dram_tensor("ones16", [1, N], F16, kind="ExternalInput").ap()
    xaug_ap = nc.dram_tensor("xaug", [N, DP], F16, kind="ExternalInput").ap()
    out_ap = nc.dram_tensor("out", [NQ, D], F32, kind="ExternalOutput").ap()

    with tile.TileContext(nc) as tc, ExitStack() as ctx:
        const = ctx.enter_context(tc.tile_pool(name="const", bufs=1))
        big = ctx.enter_context(tc.tile_pool(name="big", bufs=1))
        work = ctx.enter_context(tc.tile_pool(name="work", bufs=4))
        # PSUM budget (8 banks): ps1 [128,1024] x2 = 4, mm512 [128,512] x3 = 3,
        # po [65,512] x1 = 1.  Warmup junk MMs rotate through mm512 slots
        # before any real use.
        pp1 = ctx.enter_context(tc.tile_pool(name="pp1", bufs=2, space="PSUM"))
        pp = ctx.enter_context(tc.tile_pool(name="pp", bufs=3, space="PSUM"))
        pacc = ctx.enter_context(tc.tile_pool(name="pacc", bufs=1, space="PSUM"))

        # ---------------- PE warmup ----------------
        junk = const.tile([128, 640], F16, name="junk")
        nc.vector.memset(junk[:], 0.125)
        for w in range(WARM):
            jp = pp.tile([128, 512], F32, tag="mm512", name="jp")
            nc.tensor.matmul(jp[:], junk[:, 0:128], junk[:, 128:640],
                             start=True, stop=True)

        # ---------------- input loads ----------------
        # scalar HW queue: q-side + small tensors (parallel to sync's xT).
        xq_sb = big.tile([128, NQ], F16, name="xq_hl")
        nc.scalar.dma_start(xq_sb[:, 0:512], xqhl_ap[:, 0:512])
        rhh_sb = const.tile([128, D], F16)
        rlo_sb = const.tile([D, D], F16)
        ehh_sb = const.tile([128, D], F16)
        elo_sb = const.tile([D, D], F16)
        nc.scalar.dma_start(rhh_sb[:], rhh_ap[:])
        nc.scalar.dma_start(rlo_sb[:], rlo_ap[:])
        nc.scalar.dma_start(xq_sb[:, 512:1024], xqhl_ap[:, 512:1024])
        ident = const.tile([128, 128], F32)
        nc.scalar.dma_start(ident[:], id_ap[:])

        # sync HW queue: the big x stream.
        xt_sb = big.tile([128, N], F16, name="xt_hl")
        nc.sync.dma_start(ehh_sb[:], ehh_ap[:])
        nc.sync.dma_start(elo_sb[:], elo_ap[:])
        nc.sync.dma_start(xt_sb[:, 0:512], xhl_ap[:, 0:512])
        nc.sync.dma_start(xt_sb[:, 512:1024], xhl_ap[:, 512:1024])
        for s in range(1, 8):
            nc.sync.dma_start(xt_sb[:, s * 1024:(s + 1) * 1024],
                              xhl_ap[:, s * 1024:(s + 1) * 1024])

        # K tiles declared here so the ones rows issue on GpSimd early.
        kt_hl = [big.tile([128, KW], F16, name=f"kt_hl{q}") for q in range(NKQ)]
        kt_ss = [big.tile([DA, KW], F16, name=f"kt_ss{q}") for q in range(NKQ)]
        for q in range(NKQ):
            qw = slice(q * KW, (q + 1) * KW)
            nc.gpsimd.dma_start(kt_ss[q][D:DA, :].bitcast(F32),
                                ones16_ap[:, qw].bitcast(F32))

        # x with ones column for the PV matmul, 4 pieces for incremental
        # availability.
        xaug_r = big.tile([128, NKB * DP], F16)
        xaug_v3 = xaug_r[:].rearrange("p (t d) -> p t d", d=DP)
        xaug_src = xaug_ap.rearrange("(t p) d -> p t d", p=128)
        for s in range(4):
            t0, t1 = s * 16, (s + 1) * 16
            nc.gpsimd.dma_start(xaug_v3[:, t0:t1, :], xaug_src[:, t0:t1, :])
        xaug_v = xaug_v3[:, :, 0:DA]

        # ---------------- projections + hi/lo split ----------------
        # Qs^T first so pass-1 lhsT is ready early.
        qst_hh = big.tile([128, NQ], F16, name="qst_hh")   # [Q_hi; Q_hi]
        qst_l = big.tile([DA, NQ], F16, name="qst_l")      # [Q_lo; -m]
        q32 = big.tile([D, NQ], F32, name="q32")           # fp32 Q scratch
        for s in range(NQ // 512):
            sl = slice(s * 512, (s + 1) * 512)
            pq_full = pp.tile([128, 512], F32, tag="mm512", name="pq")
            pq = pq_full[0:D, :]
            nc.tensor.matmul(pq[:], rhh_sb[:], xq_sb[:, sl],
                             start=True, stop=False)
            nc.tensor.matmul(pq[:], rlo_sb[:], xq_sb[0:D, sl],
                             start=False, stop=True)
            nc.scalar.copy(qst_hh[0:D, sl], pq[:])
            nc.scalar.copy(q32[:, sl], pq[:])
            nc.sync.dma_start(qst_hh[D:2 * D, sl], qst_hh[0:D, sl])

        # K^T in 8 tiles of 1024 keys.
        k32 = big.tile([D, N], F32, name="k32")            # fp32 K scratch
        for s in range(NSC):
            kq, so = divmod(s, NSC // NKQ)
            sl = slice(so * 512, (so + 1) * 512)
            xsl = slice(s * 512, (s + 1) * 512)
            pk_full = pp.tile([128, 512], F32, tag="mm512", name="pk")
            pk = pk_full[0:D, :]
            nc.tensor.matmul(pk[:], ehh_sb[:], xt_sb[:, xsl],
                             start=True, stop=False)
            nc.tensor.matmul(pk[:], elo_sb[:], xt_sb[0:D, xsl],
                             start=False, stop=True)
            nc.scalar.copy(kt_ss[kq][0:D, sl], pk[:])
            nc.scalar.copy(k32[:, xsl], pk[:])
            nc.sync.dma_start(kt_hl[kq][0:D, sl], kt_ss[kq][0:D, sl])

        # lo-residual subtracts (DVE, from SBUF scratch), deferred so the
        # pass-1 reduce stream isn't blocked.
        def emit_qsubs():
            for s in range(NQ // 512):
                sl = slice(s * 512, (s + 1) * 512)
                nc.vector.tensor_tensor(
                    out=qst_l[0:D, sl], in0=q32[:, sl], in1=qst_hh[0:D, sl],
                    op=mybir.AluOpType.subtract)

        def emit_ksub(s):
            kq, so = divmod(s, NSC // NKQ)
            sl = slice(so * 512, (so + 1) * 512)
            xsl = slice(s * 512, (s + 1) * 512)
            nc.vector.tensor_tensor(
                out=kt_hl[kq][D:2 * D, sl], in0=k32[:, xsl],
                in1=kt_ss[kq][0:D, sl], op=mybir.AluOpType.subtract)

        # -------- pass 1 for chunk 0, then pass 2 per chunk with the next
        # chunk's pass 1 interleaved into the j-loop.
        mx_tiles = {}
        mxp_tiles = {}

        def emit_pass1_group(qc, gi):
            rt, g = divmod(gi, NG)
            if g == 0:
                mxp_tiles[qc] = work.tile([128, NG], F32, tag="mxp", name="mxp")
            mxp = mxp_tiles[qc]
            q0 = qc * QC + rt * 128
            ps1 = pp1.tile([128, 1024], F32, tag="ps1", name="ps1")
            for h in range(2):
                nc.tensor.matmul(ps1[:, h * 512:(h + 1) * 512],
                                 qst_hh[0:D, q0:q0 + 128],
                                 kt_ss[g][0:D, h * 512:(h + 1) * 512],
                                 start=True, stop=True)
            nc.vector.reduce_max(mxp[:, g:g + 1], ps1[:],
                                 axis=mybir.AxisListType.X)
            if g == NG - 1:
                mxt = work.tile([128, 32], F32, tag="mx_rt", name="mx_rt")
                nc.vector.memset(mxt[:], 0.0)
                nc.vector.reduce_max(mxt[:, 0:1], mxp[:],
                                     axis=mybir.AxisListType.X, negate=True)
                mx_tiles[(qc, rt)] = mxt

        def emit_max_writeback(qc):
            for rt in range(NRT):
                pm_full = pp.tile([128, 512], F32, tag="mm512", name="pm")
                ps_m = pm_full[0:32, 0:128]
                nc.tensor.transpose(ps_m[:], mx_tiles[(qc, rt)][:, 0:32],
                                    ident[:])
                sl = slice(qc * QC + rt * 128, qc * QC + (rt + 1) * 128)
                nc.vector.tensor_copy(qst_l[D:DA, sl], ps_m[0:1, :])

        emit_qsubs()
        for s in range(4):
            emit_ksub(s)
        for gi in range(NRT * NG):
            emit_pass1_group(0, gi)
        emit_max_writeback(0)
        for s in range(4, NSC):
            emit_ksub(s)

        def make_normalize(qc, po):
            def norm():
                for h in range(QC // 128):
                    ot = work.tile([DA, 128], F32, tag="ot4")
                    nc.vector.tensor_copy(ot[:], po[:, h * 128:(h + 1) * 128])
                    ptr_full = pp.tile([128, 512], F32, tag="mm512", name="ptr")
                    ps_t = ptr_full[:, 0:DA]
                    nc.tensor.transpose(ps_t[:], ot[:], ident[0:DA, 0:DA])
                    recip = work.tile([128, 1], F32, tag="recip")
                    nc.vector.reciprocal(recip[:], ps_t[:, D:DA])
                    o_sb = work.tile([128, D], F32, tag="o_sb")
                    nc.vector.tensor_scalar_mul(o_sb[:], ps_t[:, 0:D], recip[:])
                    r0 = qc * QC + h * 128
                    eng = nc.scalar if (qc == NQC - 1 and h >= 2) else nc.sync
                    eng.dma_start(out_ap[r0:r0 + 128, :], o_sb[:])
            return norm

        P1_SPREAD = 52
        p1_sched = {}
        for gi in range(NRT * NG):
            p1_sched.setdefault(gi * P1_SPREAD // (NRT * NG), []).append(gi)

        prev_norm = None
        for qc in range(NQC):
            po = pacc.tile([DA, QC], F32, tag="po")

            def emit_st(j):
                ps = pp.tile([128, QC], F32, tag="mm512", name="ps_st")
                kq, jo = divmod(j, NKB // NKQ)
                blk = slice(jo * 128, (jo + 1) * 128)
                qsl = slice(qc * QC, (qc + 1) * QC)
                nc.tensor.matmul(ps[:], kt_hl[kq][:, blk], qst_hh[:, qsl],
                                 start=True, stop=False)
                nc.tensor.matmul(ps[:], kt_ss[kq][:, blk], qst_l[:, qsl],
                                 start=False, stop=True)
                return ps

            ps_q = [emit_st(0), emit_st(1)]
            for j in range(NKB):
                pt = work.tile([128, QC], F16, tag="pt")
                nc.scalar.activation(pt[:], ps_q.pop(0)[:],
                                     mybir.ActivationFunctionType.Exp)
                if j + 2 < NKB:
                    ps_q.append(emit_st(j + 2))
                nc.tensor.matmul(po[:], xaug_v[:, j, :], pt[:],
                                 start=(j == 0), stop=(j == NKB - 1))
                if j == 3 and prev_norm is not None:
                    prev_norm()
                    prev_norm = None
                if qc + 1 < NQC:
                    for gi in p1_sched.get(j, []):
                        emit_pass1_group(qc + 1, gi)
                    if j == P1_SPREAD + 1:
                        emit_max_writeback(qc + 1)
            prev_norm = make_normalize(qc, po)
        prev_norm()

    nc.compile()
    return nc


_CACHE = {}


def _get_nc():
    if "nc" not in _CACHE:
        _CACHE["nc"] = build()
    return _CACHE["nc"]


def _hl_split(a):
    hi = a.astype(np.float16)
    lo = (a - hi.astype(np.float32)).astype(np.float16)
    return hi, lo


def kernel(x, rotation_params, entangle_params, _trace=False, _nc=None):
    from concourse.bass_utils import run_bass_kernel_spmd

    x = np.ascontiguousarray(x, dtype=np.float32)
    r8 = np.ascontiguousarray(rotation_params, dtype=np.float32) / 8.0
    e = np.ascontiguousarray(entangle_params, dtype=np.float32)

    xhi, xlo = _hl_split(x.T)
    xT_hl = np.ascontiguousarray(np.vstack([xhi, xlo]))
    rhi, rlo = _hl_split(r8)
    ehi, elo = _hl_split(e)
    rhh = np.ascontiguousarray(np.vstack([rhi, rhi]))
    ehh = np.ascontiguousarray(np.vstack([ehi, ehi]))

    nc = _nc if _nc is not None else _get_nc()
    ones16 = np.ones((1, N), dtype=np.float16)
    xaug16 = np.zeros((N, DP), dtype=np.float16)
    xaug16[:, :D] = x.astype(np.float16)
    xaug16[:, D] = 1.0

    in_maps = []
    for c in range(NCORES):
        in_maps.append({
            "xT_hl": xT_hl,
            "xqT_hl": np.ascontiguousarray(xT_hl[:, c * NQ:(c + 1) * NQ]),
            "Rhh": rhh,
            "Rlo": np.ascontiguousarray(rlo),
            "Ehh": ehh,
            "Elo": np.ascontiguousarray(elo),
            "ident": np.eye(128, dtype=np.float32),
            "ones16": ones16,
            "xaug": xaug16,
        })
    res = run_bass_kernel_spmd(nc, in_maps, core_ids=list(range(NCORES)),
                               trace=_trace)
    out = np.concatenate([res.results[c]["out"] for c in range(NCORES)], axis=0)
    if _trace:
        return out, res
    return out
